# revision 46
# baseline (speedup 1.0000x reference)
"""BiDAF attention kernel for Trainium2 (8 NeuronCores, data-parallel over batch).

Problem (per full input): B=16, L=M=1024, H=128
  s  = text@tw + (mod@mw).T + (text*tmw)@mod.T + bias          (B, L, M)
  p1 = softmax_M(mmask*s + (1-mmask)*NEG)
  p2 = softmax_L(tmask*s + (1-tmask)*NEG)
  a  = p1 @ mod
  b  = p1 @ p2.T @ text        (computed as p1 @ (p2.T @ text))
  out = [text, a, text*a, text*b]                               (B, L, 4H)

Key facts used:
  * softmax_M is invariant to per-row (per-l) shifts: s0 & bias drop from p1.
  * softmax_L is invariant to per-column (per-m) shifts: s1 & bias drop from p2.
  * masking with {0,1} is equivalent to adding (mask-1)*30000 before exp.
  * a ones-column appended to the rhs of the p1/p2 contraction matmuls
    yields the softmax denominators for free (an extra output column).
  * the s-matmul operands are bf16; the p1 numerators (E1T) are stored
    f8e5 and the final [mod|wq|1] rhs f8e4 so the output matmuls run in
    fp8 DoubleRow mode (PSUM accumulation and normalization stay f32).
  * sparsity: masked m contribute exactly 0 to p1 (masked l to p2), so both
    spaces are compacted to the unmasked rows (host-computed permutation).
  * the l-permutation is interleaved so that position p*LT+o <-> gathered
    index o*128+p; then the first LU 128-column blocks of the transposed
    text operand ARE the gathered rows, so the p2 (E2) matmul reuses the
    same operands as the p1 (E1T) matmul with no on-device gather.
  * host precomputes s0/s1 row-dots, mask biases, bf16 casts and both
    operand transposes; the device runs only 4 matmul groups + exp.
  * matmul operands are fused into one contiguous DRAM tensor per batch
    (single DMA trigger, full-line descriptors) so compute starts early.

Each of the 8 cores processes 2 batch items; no cross-core communication.
"""

import numpy as np

B, L, M, H = 16, 1024, 1024, 128
NCORES = 8
BPC = B // NCORES  # batches per core
P = 128
LT = L // P
NEGB = 30000.0

_CACHE = {}


def _build(MU, LU):
    """Per-core Bass program for MU gathered m-chunks and LU gathered
    l-chunks (SPMD: same NEFF on all 8 cores)."""
    from contextlib import ExitStack

    import concourse.bass as bass
    import concourse.mybir as mybir
    import concourse.tile as tile
    from concourse import bacc
    from concourse.bass import ts

    f32 = mybir.dt.float32
    bf16 = mybir.dt.bfloat16
    f8e4 = mybir.dt.float8e4
    f8e5 = mybir.dt.float8e5
    Exp = mybir.ActivationFunctionType.Exp
    Alu = mybir.AluOpType
    DR = mybir.MatmulPerfMode.DoubleRow

    MG = MU * P
    NE2 = [min(512, MG - i * 512) for i in range((MG + 511) // 512)]
    NQ2 = LU * (H + 1)
    WQW = 272  # modwq row, padded to a 16-byte multiple for DoubleRow

    nc = bacc.Bacc(name="bidaf8")
    # ops: [modTg (MG) | txtTs (L)] bf16 — the matmul operands laid out so
    # that [modTg | gathered txtTs] is one contiguous prefix (single early
    # DMA gates the first E2 matmul)
    ops_d = nc.dram_tensor("ops", (BPC, P, L + MG), bf16, kind="ExternalInput").ap()
    # aux: txtq2 (LU*(H+1)) bf16 — q2 rhs with ones column
    aux_d = nc.dram_tensor("aux", (BPC, P, NQ2), bf16, kind="ExternalInput").ap()
    # wqa: [mod | 0 | 1 | pad] f8e4 — final rhs, wq written on device
    wqa_d = nc.dram_tensor("wqa", (BPC, P, MU, WQW), f8e4,
                           kind="ExternalInput").ap()
    # biases, both batches: [b0: bias2|bias1, b1: bias2|bias1] f32
    bias_d = nc.dram_tensor("biases", (P, BPC * (LU + MU)), f32,
                            kind="ExternalInput").ap()
    txt_d = nc.dram_tensor("txt", (BPC, P, LT, H), f32, kind="ExternalInput").ap()
    out = nc.dram_tensor("out", (BPC, L, 4 * H), f32, kind="ExternalOutput").ap()

    with tile.TileContext(nc) as tc, ExitStack() as ctx:
        io = ctx.enter_context(tc.tile_pool(name="io", bufs=2))
        big = ctx.enter_context(tc.tile_pool(name="big", bufs=2))
        small = ctx.enter_context(tc.tile_pool(name="small", bufs=2))
        outp = ctx.enter_context(tc.tile_pool(name="outp", bufs=8))
        ps_big = ctx.enter_context(tc.tile_pool(name="ps_big", bufs=2, space="PSUM"))
        ps_fin = ctx.enter_context(tc.tile_pool(name="ps_fin", bufs=4, space="PSUM"))

        # warm the exp table while the input DMA streams: a dummy activation
        # on a memset tile makes walrus place ACT_TABLE_LOAD at kernel start
        # instead of fused in front of the first real exp's semaphore wait
        scr = small.tile([P, 1], f32, tag="scr", name="scr")
        nc.vector.memset(scr, 0.0)
        scr2 = small.tile([P, 1], f32, tag="scr2", name="scr2")
        nc.scalar.activation(scr2, scr, Exp, bias=0.0, scale=1.0)

        st = []
        # ---- loads, all triggered on sync in consumption-priority order
        # (the ring FIFO preserves trigger order, so the matmul operands of
        # batch 0 land first and compute starts as early as possible) ----
        bia = small.tile([P, BPC * (LU + MU)], f32, tag="bia", name="bia")
        for b in range(BPC):
            d = {}
            st.append(d)
            d["b2"] = bia[:, b * (LU + MU) : b * (LU + MU) + LU]
            d["b1"] = bia[:, b * (LU + MU) + LU : (b + 1) * (LU + MU)]
            d["ops"] = io.tile([P, L + MG], bf16, tag="ops", name="ops")
            d["modTg"] = d["ops"][:, :MG]
            d["txtTs"] = d["ops"][:, MG:]
            d["aux"] = io.tile([P, NQ2], bf16, tag="aux", name="aux")
            d["txtq2"] = d["aux"].rearrange("p (c h) -> p c h", h=H + 1)
            d["modwq"] = io.tile([P, MU, WQW], f8e4, tag="modwq", name="modwq")
            d["txt"] = io.tile([P, LT, H], f32, tag="txt", name="txt")
        LG = LU * P
        nc.sync.dma_start(st[0]["modTg"], ops_d[0][:, :MG])
        nc.sync.dma_start(st[0]["txtTs"][:, :LG], ops_d[0][:, MG : MG + LG])
        nc.sync.dma_start(bia, bias_d)
        nc.sync.dma_start(st[0]["aux"], aux_d[0])
        if LG < L:
            nc.sync.dma_start(st[0]["ops"][:, MG + LG :], ops_d[0][:, MG + LG :])
        nc.sync.dma_start(st[1]["ops"], ops_d[1])
        nc.sync.dma_start(st[0]["modwq"], wqa_d[0])
        nc.sync.dma_start(st[0]["txt"], txt_d[0])
        nc.sync.dma_start(st[1]["aux"], aux_d[1])
        nc.sync.dma_start(st[1]["modwq"], wqa_d[1])
        nc.sync.dma_start(st[1]["txt"], txt_d[1])

        def e2_phase(d):
            # E2[lg, mg] = exp(s2g + bias2[lg])  (p2 numerators)
            E2 = big.tile([P, LU, MG], bf16, tag="E2", name="E2")
            for c in range(LU):
                sp = ps_big.tile([P, 1024], f32, tag="s", name="sp")
                for hi, n in enumerate(NE2):
                    nc.tensor.matmul(sp[:, hi * 512 : hi * 512 + n],
                                     d["txtTs"][:, ts(c, P)],
                                     d["modTg"][:, hi * 512 : hi * 512 + n],
                                     start=True, stop=True)
                nc.scalar.activation(E2[:, c, :], sp[:, :MG], Exp,
                                     bias=d["b2"][:, c : c + 1], scale=1.0)
            d["E2"] = E2

        def e1t_phase(d):
            # E1T[mg, l] = exp(s2T + bias1[mg])  (p1 numerators), stored
            # f8e5 so the final matmuls run in fp8 DoubleRow mode
            E1T = big.tile([P, MU, L], f8e5, tag="E1T", name="E1T")
            for k in range(MU):
                sp = ps_big.tile([P, 1024], f32, tag="s", name="sp")
                for half in range(2):
                    nc.tensor.matmul(sp[:, ts(half, 512)], d["modTg"][:, ts(k, P)],
                                     d["txtTs"][:, ts(half, 512)],
                                     start=True, stop=True)
                nc.scalar.activation(E1T[:, k, :], sp, Exp,
                                     bias=d["b1"][:, k : k + 1], scale=1.0)
            d["E1T"] = E1T

        def q2_phase(d):
            # wq[mg,:] = (E2.T @ [txt|1]) / D2
            for k in range(MU):
                qp = ps_fin.tile([P, 2 * H + 1], f32, tag="pa", name="qp")
                for c in range(LU):
                    nc.tensor.matmul(qp[:, : H + 1], d["E2"][:, c, ts(k, P)],
                                     d["txtq2"][:, c, :],
                                     start=(c == 0), stop=(c == LU - 1))
                rec2 = small.tile([P, 1], f32, tag="rec2", name="rec2")
                nc.vector.reciprocal(rec2, qp[:, H : H + 1])
                nc.vector.tensor_scalar_mul(d["modwq"][:, k, H : 2 * H],
                                            qp[:, :H], rec2)

        Copy = mybir.ActivationFunctionType.Copy

        def final_phase(b, d, scalar_assist):
            # [a_raw | b_raw | D1] = E1 @ [mod | wq | 1]; k-outer waves of
            # 2 j-tiles (2 waves in flight with 4 PSUM slots) so the
            # matmuls pipeline behind the E1T exps and the assemblies.
            # scalar_assist: route the a/D1 scale through the (by then idle)
            # scalar engine to shorten the vector-bound tail.
            for w in range(4):
                js = range(2 * w, 2 * w + 2)
                pas, os_ = {}, {}
                for j in js:
                    if scalar_assist and w == 2:
                        # the big-psum slots are free once the last exps
                        # retire; using them here lets the tail waves run
                        # without waiting on earlier waves' assemblies
                        sp = ps_big.tile([P, 1024], f32, tag="s", name="pa")
                        pas[j] = sp[:, : 2 * H + 1]
                    else:
                        pas[j] = ps_fin.tile([P, 2 * H + 1], f32, tag="pa",
                                             name="pa")
                    os_[j] = outp.tile([P, 4 * H], f32, tag="o", name="o")
                    # vector, not gpsimd: keeping the Q7 engine completely
                    # idle makes the end-of-kernel gpsimd drain cheap
                    nc.vector.tensor_copy(os_[j][:, 0:H], d["txt"][:, j, :])
                NC = 2 * H + 1
                for kp in range(0, MU - 1, 2):
                    last = kp + 2 >= MU
                    for j in js:
                        nc.tensor.matmul(pas[j],
                                         d["E1T"][:, kp : kp + 2, ts(j, P)],
                                         d["modwq"][:, kp : kp + 2, :NC],
                                         perf_mode=DR,
                                         start=(kp == 0), stop=last)
                if MU % 2:
                    for j in js:
                        nc.tensor.matmul(pas[j], d["E1T"][:, MU - 1, ts(j, P)],
                                         d["modwq"][:, MU - 1, :NC],
                                         start=(MU == 1), stop=True)
                for j in js:
                    pa, o = pas[j], os_[j]
                    rec1 = small.tile([P, 1], f32, tag="rec1", name="rec1")
                    nc.vector.reciprocal(rec1, pa[:, 2 * H : 2 * H + 1])
                    if scalar_assist:
                        nc.scalar.activation(o[:, H : 2 * H], pa[:, 0:H], Copy,
                                             scale=rec1)
                    else:
                        nc.vector.tensor_scalar_mul(o[:, H : 2 * H],
                                                    pa[:, 0:H], rec1)
                    txtb = d["txt"][:, j, None, :].to_broadcast((P, 2, H))
                    nc.vector.scalar_tensor_tensor(
                        out=o[:, 2 * H :].rearrange("p (c h) -> p c h", h=H),
                        in0=pa[:, : 2 * H].rearrange("p (c h) -> p c h", h=H),
                        scalar=rec1, in1=txtb, op0=Alu.mult, op1=Alu.mult)
                    nc.sync.dma_start(
                        out[b].rearrange("(p o) c -> p o c", p=P)[:, j, :], o)

        e2_phase(st[0])
        e1t_phase(st[0])
        q2_phase(st[0])
        e2_phase(st[1])
        e1t_phase(st[1])
        final_phase(0, st[0], scalar_assist=False)
        q2_phase(st[1])
        final_phase(1, st[1], scalar_assist=True)
    nc.compile()
    return nc


def get_nc(MU, LU):
    key = (MU, LU)
    if key not in _CACHE:
        _CACHE[key] = _build(MU, LU)
    return _CACHE[key]


def make_in_maps(text, modality, text_mask, modality_mask,
                 text_weight, modality_weight, text_modality_weight):
    import ml_dtypes
    bf16 = ml_dtypes.bfloat16
    f8e4 = ml_dtypes.float8_e4m3

    text = np.asarray(text, dtype=np.float32)
    modality = np.asarray(modality, dtype=np.float32)
    tmask = np.asarray(text_mask).astype(np.int32)
    mmask = np.asarray(modality_mask).astype(np.int32)
    wt = np.asarray(text_weight, dtype=np.float32).reshape(H)
    wm = np.asarray(modality_weight, dtype=np.float32).reshape(H)
    wtm = np.asarray(text_modality_weight, dtype=np.float32).reshape(H)

    LU = max(1, int(-(-int(tmask.sum(1).max()) // P)))
    MU = max(1, int(-(-int(mmask.sum(1).max()) // P)))
    MG = MU * P
    NQ2 = LU * (H + 1)
    WQW = 272

    s0 = text @ wt        # (B, L)
    s1 = modality @ wm    # (B, M)

    # interleaved position map: gathered index i lives at position
    # (i % 128) * LT + i // 128, so position-chunk o == gathered-chunk o
    ar = np.arange(L)
    pos = (ar % P) * LT + ar // P

    in_maps = []
    row_maps = np.empty((B, L), np.int64)
    for g in range(B):
        perm_l = np.argsort(1 - tmask[g], kind="stable")
        row_maps[g][pos] = perm_l  # device position q holds original row
    for c in range(NCORES):
        txt_p = np.empty((BPC, P, LT, H), np.float32)
        ops = np.empty((BPC, P, L + MG), bf16)
        aux = np.zeros((BPC, P, NQ2), bf16)
        wqa = np.zeros((BPC, P, MU, WQW), f8e4)
        biases = np.empty((P, BPC * (LU + MU)), np.float32)
        for b in range(BPC):
            g = BPC * c + b
            perm_l = row_maps[g][pos]  # gathered order
            perm_m = np.argsort(1 - mmask[g], kind="stable")
            tg = text[g][perm_l]                      # (L, H) gathered order
            txt_p[b] = text[g][row_maps[g]].reshape(P, LT, H)
            mg_rows = modality[g][perm_m[:MG]]        # (MG, H)
            ops[b, :, :MG] = mg_rows.T                # [modTg | txtTs]
            ops[b, :, MG:] = (tg * wtm).T
            a2 = aux[b].reshape(P, LU, H + 1)
            a2[:, :, :H] = tg[: LU * P].reshape(LU, P, H).transpose(1, 0, 2)
            a2[:, :, H] = 1.0
            wqa[b, :, :, :H] = mg_rows.reshape(MU, P, H).transpose(1, 0, 2)
            wqa[b, :, :, 2 * H] = 1.0
            o = b * (LU + MU)
            biases[:, o : o + LU] = (s0[g][perm_l[: LU * P]]
                                     + (tmask[g][perm_l[: LU * P]] - 1.0) * NEGB
                                     ).reshape(LU, P).T
            biases[:, o + LU : o + LU + MU] = (s1[g][perm_m[:MG]]
                                               + (mmask[g][perm_m[:MG]] - 1.0)
                                               * NEGB).reshape(MU, P).T
        in_maps.append({
            "ops": ops, "aux": aux, "wqa": wqa, "biases": biases, "txt": txt_p,
        })
    return in_maps, row_maps, MU, LU


def kernel(text, modality, text_mask, modality_mask,
           text_weight, modality_weight, text_modality_weight, bias,
           trace=False):
    from concourse.bass_utils import run_bass_kernel_spmd

    in_maps, row_maps, MU, LU = make_in_maps(
        text, modality, text_mask, modality_mask,
        text_weight, modality_weight, text_modality_weight)
    nc = get_nc(MU, LU)
    res = run_bass_kernel_spmd(nc, in_maps, core_ids=list(range(NCORES)),
                               trace=trace)
    outp = np.empty((B, L, 4 * H), np.float32)
    for c in range(NCORES):
        dev = res.results[c]["out"]
        for b in range(BPC):
            g = BPC * c + b
            outp[g][row_maps[g]] = dev[b]
    if trace:
        kernel.last_result = res
    return outp


# revision 48
# speedup vs baseline: 1.0655x; 1.0655x over previous
"""BiDAF attention kernel for Trainium2 (8 NeuronCores, data-parallel over batch).

Problem (per full input): B=16, L=M=1024, H=128
  s  = text@tw + (mod@mw).T + (text*tmw)@mod.T + bias          (B, L, M)
  p1 = softmax_M(mmask*s + (1-mmask)*NEG)
  p2 = softmax_L(tmask*s + (1-tmask)*NEG)
  a  = p1 @ mod
  b  = p1 @ p2.T @ text        (computed as p1 @ (p2.T @ text))
  out = [text, a, text*a, text*b]                               (B, L, 4H)

Key facts used:
  * softmax_M is invariant to per-row (per-l) shifts: s0 & bias drop from p1.
  * softmax_L is invariant to per-column (per-m) shifts: s1 & bias drop from p2.
  * masking with {0,1} is equivalent to adding (mask-1)*30000 before exp.
  * a ones-column appended to the rhs of the p1/p2 contraction matmuls
    yields the softmax denominators for free (an extra output column).
  * the s-matmul operands are bf16; the p1 numerators (E1T) are stored
    f8e5 and the final [mod|wq|1] rhs f8e4 so the output matmuls run in
    fp8 DoubleRow mode (PSUM accumulation and normalization stay f32).
  * sparsity: masked m contribute exactly 0 to p1 (masked l to p2), so both
    spaces are compacted to the unmasked rows (host-computed permutation).
  * the l-permutation is interleaved so that position p*LT+o <-> gathered
    index o*128+p; then the first LU 128-column blocks of the transposed
    text operand ARE the gathered rows, so the p2 (E2) matmul reuses the
    same operands as the p1 (E1T) matmul with no on-device gather.
  * host precomputes s0/s1 row-dots, mask biases, bf16 casts and both
    operand transposes; the device runs only 4 matmul groups + exp.
  * matmul operands are fused into one contiguous DRAM tensor per batch
    (single DMA trigger, full-line descriptors) so compute starts early.

Each of the 8 cores processes 2 batch items; no cross-core communication.
"""

import numpy as np

B, L, M, H = 16, 1024, 1024, 128
NCORES = 8
BPC = B // NCORES  # batches per core
P = 128
LT = L // P
NEGB = 30000.0

_CACHE = {}


def _build(MU, LU):
    """Per-core Bass program for MU gathered m-chunks and LU gathered
    l-chunks (SPMD: same NEFF on all 8 cores)."""
    from contextlib import ExitStack

    import concourse.bass as bass
    import concourse.mybir as mybir
    import concourse.tile as tile
    from concourse import bacc
    from concourse.bass import ts

    f32 = mybir.dt.float32
    bf16 = mybir.dt.bfloat16
    f8e4 = mybir.dt.float8e4
    f8e5 = mybir.dt.float8e5
    Exp = mybir.ActivationFunctionType.Exp
    Alu = mybir.AluOpType
    DR = mybir.MatmulPerfMode.DoubleRow

    MG = MU * P
    NE2 = [min(512, MG - i * 512) for i in range((MG + 511) // 512)]
    NQ2 = LU * (H + 1)
    WQW = 272  # modwq row, padded to a 16-byte multiple for DoubleRow

    nc = bacc.Bacc(name="bidaf8")
    # ops: [modTg (MG) | txtTs (L)] bf16 — the matmul operands laid out so
    # that [modTg | gathered txtTs] is one contiguous prefix (single early
    # DMA gates the first E2 matmul)
    ops_d = nc.dram_tensor("ops", (BPC, P, L + MG), bf16, kind="ExternalInput").ap()
    # aux: txtq2 (LU*(H+1)) bf16 — q2 rhs with ones column
    aux_d = nc.dram_tensor("aux", (BPC, P, NQ2), bf16, kind="ExternalInput").ap()
    # wqa: [mod | 0 | 1 | pad] f8e4 — final rhs, wq written on device
    wqa_d = nc.dram_tensor("wqa", (BPC, P, MU, WQW), f8e4,
                           kind="ExternalInput").ap()
    # biases, both batches: [b0: bias2|bias1, b1: bias2|bias1] f32
    bias_d = nc.dram_tensor("biases", (P, BPC * (LU + MU)), f32,
                            kind="ExternalInput").ap()
    txt_d = nc.dram_tensor("txt", (BPC, P, LT, H), f32, kind="ExternalInput").ap()
    out = nc.dram_tensor("out", (BPC, L, 4 * H), f32, kind="ExternalOutput").ap()

    with tile.TileContext(nc) as tc, ExitStack() as ctx:
        io = ctx.enter_context(tc.tile_pool(name="io", bufs=2))
        big = ctx.enter_context(tc.tile_pool(name="big", bufs=2))
        small = ctx.enter_context(tc.tile_pool(name="small", bufs=2))
        # 16 slots = both batches' output tiles live at once, so batch 1's
        # assembly never waits on batch 0's store DMAs draining the ring
        outp = ctx.enter_context(tc.tile_pool(name="outp", bufs=16))
        ps_big = ctx.enter_context(tc.tile_pool(name="ps_big", bufs=2, space="PSUM"))
        ps_fin = ctx.enter_context(tc.tile_pool(name="ps_fin", bufs=4, space="PSUM"))

        # warm the exp table while the input DMA streams: a dummy activation
        # on a memset tile makes walrus place ACT_TABLE_LOAD at kernel start
        # instead of fused in front of the first real exp's semaphore wait
        scr = small.tile([P, 1], f32, tag="scr", name="scr")
        nc.vector.memset(scr, 0.0)
        scr2 = small.tile([P, 1], f32, tag="scr2", name="scr2")
        nc.scalar.activation(scr2, scr, Exp, bias=0.0, scale=1.0)

        st = []
        # ---- loads, all triggered on sync in consumption-priority order
        # (the ring FIFO preserves trigger order, so the matmul operands of
        # batch 0 land first and compute starts as early as possible) ----
        bia = small.tile([P, BPC * (LU + MU)], f32, tag="bia", name="bia")
        for b in range(BPC):
            d = {}
            st.append(d)
            d["b2"] = bia[:, b * (LU + MU) : b * (LU + MU) + LU]
            d["b1"] = bia[:, b * (LU + MU) + LU : (b + 1) * (LU + MU)]
            d["ops"] = io.tile([P, L + MG], bf16, tag="ops", name="ops")
            d["modTg"] = d["ops"][:, :MG]
            d["txtTs"] = d["ops"][:, MG:]
            d["aux"] = io.tile([P, NQ2], bf16, tag="aux", name="aux")
            d["txtq2"] = d["aux"].rearrange("p (c h) -> p c h", h=H + 1)
            d["modwq"] = io.tile([P, MU, WQW], f8e4, tag="modwq", name="modwq")
            d["txt"] = io.tile([P, LT, H], f32, tag="txt", name="txt")
        LG = LU * P
        nc.sync.dma_start(st[0]["modTg"], ops_d[0][:, :MG])
        nc.sync.dma_start(st[0]["txtTs"][:, :LG], ops_d[0][:, MG : MG + LG])
        nc.sync.dma_start(bia, bias_d)
        nc.sync.dma_start(st[0]["aux"], aux_d[0])
        if LG < L:
            nc.sync.dma_start(st[0]["ops"][:, MG + LG :], ops_d[0][:, MG + LG :])
        nc.sync.dma_start(st[1]["ops"], ops_d[1])
        nc.sync.dma_start(st[0]["modwq"], wqa_d[0])
        nc.sync.dma_start(st[0]["txt"], txt_d[0])
        nc.sync.dma_start(st[1]["aux"], aux_d[1])
        nc.sync.dma_start(st[1]["modwq"], wqa_d[1])
        nc.sync.dma_start(st[1]["txt"], txt_d[1])

        def e2_phase(d):
            # E2[lg, mg] = exp(s2g + bias2[lg])  (p2 numerators)
            E2 = big.tile([P, LU, MG], bf16, tag="E2", name="E2")
            for c in range(LU):
                sp = ps_big.tile([P, 1024], f32, tag="s", name="sp")
                for hi, n in enumerate(NE2):
                    nc.tensor.matmul(sp[:, hi * 512 : hi * 512 + n],
                                     d["txtTs"][:, ts(c, P)],
                                     d["modTg"][:, hi * 512 : hi * 512 + n],
                                     start=True, stop=True)
                nc.scalar.activation(E2[:, c, :], sp[:, :MG], Exp,
                                     bias=d["b2"][:, c : c + 1], scale=1.0)
            d["E2"] = E2

        def e1t_phase(d):
            # E1T[mg, l] = exp(s2T + bias1[mg])  (p1 numerators), stored
            # f8e5 so the final matmuls run in fp8 DoubleRow mode
            E1T = big.tile([P, MU, L], f8e5, tag="E1T", name="E1T")
            for k in range(MU):
                sp = ps_big.tile([P, 1024], f32, tag="s", name="sp")
                for half in range(2):
                    nc.tensor.matmul(sp[:, ts(half, 512)], d["modTg"][:, ts(k, P)],
                                     d["txtTs"][:, ts(half, 512)],
                                     start=True, stop=True)
                nc.scalar.activation(E1T[:, k, :], sp, Exp,
                                     bias=d["b1"][:, k : k + 1], scale=1.0)
            d["E1T"] = E1T

        def q2_phase(d):
            # wq[mg,:] = (E2.T @ [txt|1]) / D2
            for k in range(MU):
                qp = ps_fin.tile([P, 2 * H + 1], f32, tag="pa", name="qp")
                for c in range(LU):
                    nc.tensor.matmul(qp[:, : H + 1], d["E2"][:, c, ts(k, P)],
                                     d["txtq2"][:, c, :],
                                     start=(c == 0), stop=(c == LU - 1))
                rec2 = small.tile([P, 1], f32, tag="rec2", name="rec2")
                nc.vector.reciprocal(rec2, qp[:, H : H + 1])
                nc.vector.tensor_scalar_mul(d["modwq"][:, k, H : 2 * H],
                                            qp[:, :H], rec2)

        Copy = mybir.ActivationFunctionType.Copy

        def final_phase(b, d, scalar_assist):
            # [a_raw | b_raw | D1] = E1 @ [mod | wq | 1]; k-outer waves of
            # 2 j-tiles (2 waves in flight with 4 PSUM slots) so the
            # matmuls pipeline behind the E1T exps and the assemblies.
            # scalar_assist: route the a/D1 scale through the (by then idle)
            # scalar engine to shorten the vector-bound tail.
            for w in range(4):
                js = range(2 * w, 2 * w + 2)
                pas, os_ = {}, {}
                for j in js:
                    if scalar_assist and w == 2:
                        # the big-psum slots are free once the last exps
                        # retire; using them here lets the tail waves run
                        # without waiting on earlier waves' assemblies
                        sp = ps_big.tile([P, 1024], f32, tag="s", name="pa")
                        pas[j] = sp[:, : 2 * H + 1]
                    else:
                        pas[j] = ps_fin.tile([P, 2 * H + 1], f32, tag="pa",
                                             name="pa")
                    os_[j] = outp.tile([P, 4 * H], f32, tag="o", name="o")
                    nc.gpsimd.tensor_copy(os_[j][:, 0:H], d["txt"][:, j, :])
                NC = 2 * H + 1
                for kp in range(0, MU - 1, 2):
                    last = kp + 2 >= MU
                    for j in js:
                        nc.tensor.matmul(pas[j],
                                         d["E1T"][:, kp : kp + 2, ts(j, P)],
                                         d["modwq"][:, kp : kp + 2, :NC],
                                         perf_mode=DR,
                                         start=(kp == 0), stop=last)
                if MU % 2:
                    for j in js:
                        nc.tensor.matmul(pas[j], d["E1T"][:, MU - 1, ts(j, P)],
                                         d["modwq"][:, MU - 1, :NC],
                                         start=(MU == 1), stop=True)
                for j in js:
                    pa, o = pas[j], os_[j]
                    rec1 = small.tile([P, 1], f32, tag="rec1", name="rec1")
                    nc.vector.reciprocal(rec1, pa[:, 2 * H : 2 * H + 1])
                    if scalar_assist:
                        nc.scalar.activation(o[:, H : 2 * H], pa[:, 0:H], Copy,
                                             scale=rec1)
                    else:
                        nc.vector.tensor_scalar_mul(o[:, H : 2 * H],
                                                    pa[:, 0:H], rec1)
                    txtb = d["txt"][:, j, None, :].to_broadcast((P, 2, H))
                    nc.vector.scalar_tensor_tensor(
                        out=o[:, 2 * H :].rearrange("p (c h) -> p c h", h=H),
                        in0=pa[:, : 2 * H].rearrange("p (c h) -> p c h", h=H),
                        scalar=rec1, in1=txtb, op0=Alu.mult, op1=Alu.mult)
                    nc.sync.dma_start(
                        out[b].rearrange("(p o) c -> p o c", p=P)[:, j, :], o)

        e2_phase(st[0])
        e1t_phase(st[0])
        q2_phase(st[0])
        e2_phase(st[1])
        e1t_phase(st[1])
        final_phase(0, st[0], scalar_assist=False)
        q2_phase(st[1])
        final_phase(1, st[1], scalar_assist=True)
    nc.compile()
    return nc


def get_nc(MU, LU):
    key = (MU, LU)
    if key not in _CACHE:
        _CACHE[key] = _build(MU, LU)
    return _CACHE[key]


def make_in_maps(text, modality, text_mask, modality_mask,
                 text_weight, modality_weight, text_modality_weight):
    import ml_dtypes
    bf16 = ml_dtypes.bfloat16
    f8e4 = ml_dtypes.float8_e4m3

    text = np.asarray(text, dtype=np.float32)
    modality = np.asarray(modality, dtype=np.float32)
    tmask = np.asarray(text_mask).astype(np.int32)
    mmask = np.asarray(modality_mask).astype(np.int32)
    wt = np.asarray(text_weight, dtype=np.float32).reshape(H)
    wm = np.asarray(modality_weight, dtype=np.float32).reshape(H)
    wtm = np.asarray(text_modality_weight, dtype=np.float32).reshape(H)

    LU = max(1, int(-(-int(tmask.sum(1).max()) // P)))
    MU = max(1, int(-(-int(mmask.sum(1).max()) // P)))
    MG = MU * P
    NQ2 = LU * (H + 1)
    WQW = 272

    s0 = text @ wt        # (B, L)
    s1 = modality @ wm    # (B, M)

    # interleaved position map: gathered index i lives at position
    # (i % 128) * LT + i // 128, so position-chunk o == gathered-chunk o
    ar = np.arange(L)
    pos = (ar % P) * LT + ar // P

    in_maps = []
    row_maps = np.empty((B, L), np.int64)
    for g in range(B):
        perm_l = np.argsort(1 - tmask[g], kind="stable")
        row_maps[g][pos] = perm_l  # device position q holds original row
    for c in range(NCORES):
        txt_p = np.empty((BPC, P, LT, H), np.float32)
        ops = np.empty((BPC, P, L + MG), bf16)
        aux = np.zeros((BPC, P, NQ2), bf16)
        wqa = np.zeros((BPC, P, MU, WQW), f8e4)
        biases = np.empty((P, BPC * (LU + MU)), np.float32)
        for b in range(BPC):
            g = BPC * c + b
            perm_l = row_maps[g][pos]  # gathered order
            perm_m = np.argsort(1 - mmask[g], kind="stable")
            tg = text[g][perm_l]                      # (L, H) gathered order
            txt_p[b] = text[g][row_maps[g]].reshape(P, LT, H)
            mg_rows = modality[g][perm_m[:MG]]        # (MG, H)
            ops[b, :, :MG] = mg_rows.T                # [modTg | txtTs]
            ops[b, :, MG:] = (tg * wtm).T
            a2 = aux[b].reshape(P, LU, H + 1)
            a2[:, :, :H] = tg[: LU * P].reshape(LU, P, H).transpose(1, 0, 2)
            a2[:, :, H] = 1.0
            wqa[b, :, :, :H] = mg_rows.reshape(MU, P, H).transpose(1, 0, 2)
            wqa[b, :, :, 2 * H] = 1.0
            o = b * (LU + MU)
            biases[:, o : o + LU] = (s0[g][perm_l[: LU * P]]
                                     + (tmask[g][perm_l[: LU * P]] - 1.0) * NEGB
                                     ).reshape(LU, P).T
            biases[:, o + LU : o + LU + MU] = (s1[g][perm_m[:MG]]
                                               + (mmask[g][perm_m[:MG]] - 1.0)
                                               * NEGB).reshape(MU, P).T
        in_maps.append({
            "ops": ops, "aux": aux, "wqa": wqa, "biases": biases, "txt": txt_p,
        })
    return in_maps, row_maps, MU, LU


def kernel(text, modality, text_mask, modality_mask,
           text_weight, modality_weight, text_modality_weight, bias,
           trace=False):
    from concourse.bass_utils import run_bass_kernel_spmd

    in_maps, row_maps, MU, LU = make_in_maps(
        text, modality, text_mask, modality_mask,
        text_weight, modality_weight, text_modality_weight)
    nc = get_nc(MU, LU)
    res = run_bass_kernel_spmd(nc, in_maps, core_ids=list(range(NCORES)),
                               trace=trace)
    outp = np.empty((B, L, 4 * H), np.float32)
    for c in range(NCORES):
        dev = res.results[c]["out"]
        for b in range(BPC):
            g = BPC * c + b
            outp[g][row_maps[g]] = dev[b]
    if trace:
        kernel.last_result = res
    return outp
